# revision 2
# baseline (speedup 1.0000x reference)
"""Two-layer GAT on 8 Trainium2 NeuronCores (Bass/Tile, no collectives).

Strategy
--------
* Node ids are relabeled into G=4 contiguous "range groups" of GSZ rows so
  that dma_gather's int16 indices can address any node (idx local to group,
  gather base = group offset). Each group also carries one dedicated pad row
  whose attention-score field is -60000 (=> exp -> 0, messages masked).
* Destination nodes are sharded across the 8 cores (12500 each). Each core
  processes its own shard's edges; dense transforms (x@W) are computed
  redundantly on every core (cheaper than cross-core collectives here).
* Per layer there is a feature "table" in DRAM: row = [feat | a_src | a_dst]
  padded to 128 fp16 (256 B = dma_gather's min element size). The edge phase
  gathers, per 128-dst-node tile, all incoming-neighbor rows with dma_gather
  (1024 idx / instr, 4 SWDGE queues), then does the segment softmax and the
  weighted message sum on DVE with dense per-partition reductions.
* Two device launches: L1 = transform(x)->table1 + edge pass -> h2 shards
  (elu(out+b1) fused). Host reassembles h2, transposes/casts, launch 2 runs
  transform(h2)->table2 + edge pass -> final [N, 40] output.
"""
import sys
sys.path.insert(0, "/opt/trn_rl_repo")

import numpy as np

P = 128
DCHUNK = 8          # slot-columns per dma_gather (128*8 = 1024 idx)
NEG = -60000.0      # pad-row attention score

_F32 = None
_F16 = None
_I16 = None


def _mybir():
    from concourse import mybir
    return mybir


def make_cfg(n_raw=100000, f_in=512, hh=8, cc=8, out_w=40, ncores=8, gsz=25088):
    g = 4
    assert gsz <= 32768 and (n_raw + g - 1) // g + 1 <= gsz
    sh = n_raw // ncores
    ntiles = (sh + P - 1) // P
    return dict(
        N_RAW=n_raw, F_IN=f_in, HH=hh, CC=cc, F_HID=hh * cc, OUT_W=out_w,
        NCORES=ncores, SH=sh, NTILES=ntiles, SHPAD=ntiles * P,
        G=g, GSZ=gsz, PAD_LOCAL=gsz - 1, NV=g * gsz,
    )


# --------------------------------------------------------------------------
# host-side graph prep
# --------------------------------------------------------------------------

def prep_graph(cfg, src_orig, dst_orig):
    """Build per-core padded slot matrices and the uniform per-tile shapes.

    Returns (D_tbl [NTILES][G], blobs [ncores arrays [128, CB] int16],
             orow [ncores arrays [SH] original ids in processing order],
             tile_col_offsets, CB)
    """
    G, GSZ, SH, NT = cfg["G"], cfg["GSZ"], cfg["SH"], cfg["NTILES"]
    SHPAD, PAD_LOCAL, NC = cfg["SHPAD"], cfg["PAD_LOCAL"], cfg["NCORES"]

    src_orig = np.asarray(src_orig, dtype=np.int64)
    dst_orig = np.asarray(dst_orig, dtype=np.int64)

    per_core = []
    for r in range(NC):
        lo, hi = r * SH, (r + 1) * SH
        m = (dst_orig >= lo) & (dst_orig < hi)
        es, ed = src_orig[m], dst_orig[m] - lo
        sg = (es % G).astype(np.int64)
        sl = (es // G).astype(np.int16)
        key = ed * G + sg
        o = np.argsort(key, kind="stable")
        ks, vs = key[o], sl[o]
        cnt = np.bincount(ks, minlength=SH * G).reshape(SH, G)
        starts = np.zeros(SH * G, dtype=np.int64)
        np.cumsum(cnt.ravel()[:-1], out=starts[1:])
        col = np.arange(len(ks)) - starts[ks]
        per_core.append((ks, vs, col, cnt))

    dmax = max(int(pc[3].max()) for pc in per_core) + 1  # +1 slot0

    D_core = np.zeros((NC, NT, G), dtype=np.int64)
    bigs, orows = [], []
    for r in range(NC):
        ks, vs, col, cnt = per_core[r]
        big = np.full((SHPAD, G, dmax), PAD_LOCAL, dtype=np.int16)
        big[ks // G, ks % G, 1 + col] = vs
        orig_ids = np.arange(r * SH, (r + 1) * SH, dtype=np.int64)
        own_g = (orig_ids % G).astype(np.int64)
        own_l = (orig_ids // G).astype(np.int16)
        big[np.arange(SH), own_g, 0] = own_l
        slots = cnt.sum(1) + 1
        order = np.argsort(-slots, kind="stable")
        big[:SH] = big[:SH][order]
        cnt_o = cnt[order]
        orows.append(orig_ids[order])
        bigs.append(big)
        for t in range(NT):
            seg = cnt_o[t * P:(t + 1) * P]
            if len(seg):
                D_core[r, t] = 1 + seg.max(axis=0)
            else:
                D_core[r, t] = 1

    D_tbl = D_core.max(axis=0)  # [NT, G] uniform across cores

    # column offsets in the idx blob
    tile_off = []
    off = 0
    for t in range(NT):
        tile_off.append(off)
        off += 8 * int(D_tbl[t].sum())
    CB = off

    blobs = []
    for r in range(NC):
        big = bigs[r]
        blob = np.empty((P, CB), dtype=np.int16)
        for t in range(NT):
            c = tile_off[t]
            for g in range(G):
                D = int(D_tbl[t, g])
                mat = big[t * P:(t + 1) * P, g, :D]  # [128, D]
                for ck in range(0, D, DCHUNK):
                    d = min(DCHUNK, D - ck)
                    L = mat[:, ck:ck + d].T.ravel()            # i = j*128+p
                    W16 = L.reshape(-1, 16).T                  # [16, 8*d]
                    blob[:, c:c + 8 * d] = np.tile(W16, (8, 1))
                    c += 8 * d
        blobs.append(blob)

    return D_tbl, blobs, orows, tile_off, CB


# --------------------------------------------------------------------------
# device program
# --------------------------------------------------------------------------

def build_launch(cfg, layer, D_tbl, tile_off, CB):
    """layer 1: f_in=F_IN, row=[h(64)|as(8)|ad(8)], out=[SHPAD,64] (elu(x+b1))
       layer 2: f_in=F_HID, row=[g(40)|as2|ad2], out=[SHPAD,40] (x+b2)"""
    import concourse.bass as bass
    import concourse.bacc as bacc
    import concourse.tile as tile
    mybir = _mybir()
    f16, f32, i16 = mybir.dt.float16, mybir.dt.float32, mybir.dt.int32
    i16 = mybir.dt.int16

    G, GSZ, NV, NT = cfg["G"], cfg["GSZ"], cfg["NV"], cfg["NTILES"]
    SHPAD, PAD_LOCAL = cfg["SHPAD"], cfg["PAD_LOCAL"]
    HH = cfg["HH"] if layer == 1 else 1
    CC = cfg["CC"] if layer == 1 else cfg["OUT_W"]
    CT = HH * CC                       # 64 | 40
    AS_OFF, AD_OFF = CT, CT + HH       # 64,72 | 40,41
    RW = CT + 2 * HH                   # 80 | 42
    F_IN = cfg["F_IN"] if layer == 1 else cfg["F_HID"]
    OUT_W = CT

    nc = bacc.Bacc("TRN2", target_bir_lowering=False, debug=False,
                   num_swdge_queues=4)
    fT = nc.dram_tensor("fT", [F_IN, NV], f16, kind="ExternalInput")
    Wp = nc.dram_tensor("Wp", [F_IN, RW], f16, kind="ExternalInput")
    bias = nc.dram_tensor("bias", [OUT_W], f32, kind="ExternalInput")
    idxb = nc.dram_tensor("idxb", [P, CB], i16, kind="ExternalInput")
    outo = nc.dram_tensor("out", [SHPAD, OUT_W], f32, kind="ExternalOutput")
    table = nc.dram_tensor("table", [NV, 128], f16)

    KCHUNKS = [(k, min(P, F_IN - k)) for k in range(0, F_IN, P)]
    qn = [0]

    def nextq():
        qn[0] = (qn[0] + 1) % 4
        return qn[0]

    with tile.TileContext(nc) as tc:
        import contextlib
        with contextlib.ExitStack() as ctx:
            singles = ctx.enter_context(tc.tile_pool(name="singles", bufs=1))
            xtp = ctx.enter_context(tc.tile_pool(name="xt", bufs=3))
            psp = ctx.enter_context(tc.tile_pool(name="ps", bufs=4, space="PSUM"))
            otp = ctx.enter_context(tc.tile_pool(name="ot", bufs=4))
            gp = ctx.enter_context(tc.tile_pool(name="gp", bufs=2))
            ip = ctx.enter_context(tc.tile_pool(name="ip", bufs=2))
            ep = ctx.enter_context(tc.tile_pool(name="ep", bufs=2))
            sp = ctx.enter_context(tc.tile_pool(name="sp", bufs=3))
            mp = ctx.enter_context(tc.tile_pool(name="mp", bufs=2))

            # ---------------- singles ----------------
            wts = []
            for kc, (k0, kn) in enumerate(KCHUNKS):
                wt = singles.tile([P, RW], f16, tag=f"w{kc}")
                nc.sync.dma_start(out=wt[:kn, :], in_=Wp[k0:k0 + kn, :])
                wts.append(wt)
            bias_ap = bias[:]
            bias_b = bass.AP(tensor=bias_ap.tensor, offset=bias_ap.offset,
                             ap=[[0, P]] + list(bias_ap.ap))
            bt = singles.tile([P, OUT_W], f32)
            nc.sync.dma_start(out=bt[:], in_=bias_b)
            padt = singles.tile([P, RW], f16)
            nc.vector.memset(padt[:], 0.0)
            nc.vector.memset(padt[:, AS_OFF:AS_OFF + HH], NEG)

            # ---------------- transform: fT.T @ Wp -> table ----------------
            NCH = NV // 512
            for ch in range(NCH):
                xts = []
                for kc, (k0, kn) in enumerate(KCHUNKS):
                    xt = xtp.tile([P, 512], f16, tag=f"x{kc}")
                    nc.sync.dma_start(out=xt[:kn, :],
                                      in_=fT[k0:k0 + kn, ch * 512:(ch + 1) * 512])
                    xts.append(xt)
                for sub in range(4):
                    pt = psp.tile([P, RW], f32)
                    for kc, (k0, kn) in enumerate(KCHUNKS):
                        nc.tensor.matmul(
                            out=pt[:],
                            lhsT=xts[kc][:kn, sub * P:(sub + 1) * P],
                            rhs=wts[kc][:kn, :],
                            start=(kc == 0), stop=(kc == len(KCHUNKS) - 1))
                    ot = otp.tile([P, RW], f16)
                    nc.vector.tensor_copy(out=ot[:], in_=pt[:])
                    r0 = ch * 512 + sub * P
                    nc.sync.dma_start(out=table[r0:r0 + P, 0:RW], in_=ot[:])

            # pad rows (overwrite whatever the transform put there)
            for g in range(G):
                pr = g * GSZ + PAD_LOCAL
                nc.sync.dma_start(out=table[pr:pr + 1, 0:RW], in_=padt[0:1, :])

            # ---------------- edge phase ----------------
            for t in range(NT):
                Ds = [int(D_tbl[t, g]) for g in range(G)]
                SD = sum(Ds)
                tcols = 8 * SD
                it = ip.tile([P, tcols], i16, tag="idx")
                nc.sync.dma_start(
                    out=it[:], in_=idxb[:, tile_off[t]:tile_off[t] + tcols])

                Gt = gp.tile([P, SD, 128], f16, tag="G")
                c0 = 0   # slot-column offset in Gt
                ic = 0   # idx column offset in it
                g_off = []
                for g in range(G):
                    g_off.append(c0)
                    D = Ds[g]
                    for ck in range(0, D, DCHUNK):
                        d = min(DCHUNK, D - ck)
                        nc.gpsimd.dma_gather(
                            out_ap=Gt[:, c0:c0 + d, :],
                            in_ap=table[g * GSZ:, :],
                            idxs_ap=it[:, ic:ic + 8 * d],
                            num_idxs=P * d,
                            num_idxs_reg=P * d,
                            elem_size=128,
                            queue_num=nextq(),
                        )
                        c0 += d
                        ic += 8 * d

                # ad_own [128, HH] (slot0 of own group; others are pad rows=0)
                adt = sp.tile([P, HH], f16, tag="ad")
                nc.vector.tensor_copy(out=adt[:],
                                      in_=Gt[:, g_off[0], AD_OFF:AD_OFF + HH])
                for g in range(1, G):
                    nc.vector.tensor_add(out=adt[:], in0=adt[:],
                                         in1=Gt[:, g_off[g], AD_OFF:AD_OFF + HH])

                # e = leaky_relu(as_src + ad_own), layout [128, HH, SD]
                et = ep.tile([P, HH, SD], f32, tag="e")
                nc.vector.tensor_tensor(
                    out=et[:].rearrange("p h d -> p d h"),
                    in0=Gt[:, :, AS_OFF:AS_OFF + HH],
                    in1=adt[:].unsqueeze(1).broadcast_to([P, SD, HH]),
                    op=mybir.AluOpType.add)
                # leaky_relu(e) = max(e, 0.2*e)
                ef = et[:].rearrange("p h d -> p (h d)")
                lk = ep.tile([P, HH, SD], f32, tag="lk")
                lkf = lk[:].rearrange("p h d -> p (h d)")
                nc.vector.tensor_scalar_mul(out=lkf, in0=ef, scalar1=0.2)
                nc.vector.tensor_tensor(out=ef, in0=ef, in1=lkf,
                                        op=mybir.AluOpType.max)

                # m = rowmax; e -= m; ex = exp(e) (fp16, <=1)
                mt = sp.tile([P, HH], f32, tag="m")
                nc.vector.reduce_max(out=mt[:], in_=et[:],
                                     axis=mybir.AxisListType.X)
                nmt = sp.tile([P, HH], f32, tag="nm")
                nc.vector.tensor_scalar_mul(out=nmt[:], in0=mt[:], scalar1=-1.0)
                nc.vector.tensor_tensor(
                    out=et[:].rearrange("p h d -> p d h"),
                    in0=et[:].rearrange("p h d -> p d h"),
                    in1=nmt[:].unsqueeze(1).broadcast_to([P, SD, HH]),
                    op=mybir.AluOpType.add)
                ext = ep.tile([P, HH, SD], f16, tag="ex")
                nc.scalar.activation(out=ext[:].rearrange("p h d -> p (h d)"),
                                     in_=ef,
                                     func=mybir.ActivationFunctionType.Exp)

                # denom = sum ex + 1e-16 ; rden = 1/denom
                dent = sp.tile([P, HH], f32, tag="den")
                nc.vector.reduce_sum(out=dent[:], in_=ext[:],
                                     axis=mybir.AxisListType.X)
                nc.vector.tensor_scalar_add(out=dent[:], in0=dent[:],
                                            scalar1=1e-16)
                rdt = sp.tile([P, HH], f32, tag="rd")
                nc.vector.reciprocal(out=rdt[:], in_=dent[:])

                # msg = feat * ex ; out_raw = sum over slots
                mg = mp.tile([P, CT, SD], f32, tag="msg")
                nc.vector.tensor_tensor(
                    out=mg[:].rearrange("p (h c) d -> p d h c", h=HH),
                    in0=Gt[:, :, 0:CT].rearrange("p d (h c) -> p d h c", h=HH),
                    in1=ext[:].rearrange("p h d -> p d h").unsqueeze(3)
                        .broadcast_to([P, SD, HH, CC]),
                    op=mybir.AluOpType.mult)
                ort = sp.tile([P, CT], f32, tag="or")
                nc.vector.reduce_sum(out=ort[:], in_=mg[:],
                                     axis=mybir.AxisListType.X)

                # out1 = out_raw * rden + bias
                o1 = sp.tile([P, CT], f32, tag="o1")
                nc.vector.tensor_tensor(
                    out=o1[:].rearrange("p (h c) -> p h c", h=HH),
                    in0=ort[:].rearrange("p (h c) -> p h c", h=HH),
                    in1=rdt[:].unsqueeze(2).broadcast_to([P, HH, CC]),
                    op=mybir.AluOpType.mult)
                nc.vector.tensor_add(out=o1[:], in0=o1[:], in1=bt[:])

                if layer == 1:
                    # elu(x) = relu(x) + exp(min(x,0)) - 1
                    t1 = sp.tile([P, CT], f32, tag="t1")
                    nc.vector.tensor_scalar_min(out=t1[:], in0=o1[:], scalar1=0.0)
                    nc.scalar.activation(out=t1[:], in_=t1[:],
                                         func=mybir.ActivationFunctionType.Exp)
                    t2 = sp.tile([P, CT], f32, tag="t2")
                    nc.vector.tensor_scalar_max(out=t2[:], in0=o1[:], scalar1=0.0)
                    nc.vector.tensor_add(out=t1[:], in0=t1[:], in1=t2[:])
                    nc.vector.tensor_scalar_add(out=t1[:], in0=t1[:], scalar1=-1.0)
                    res = t1
                else:
                    res = o1
                nc.sync.dma_start(out=outo[t * P:(t + 1) * P, :], in_=res[:])

    nc.compile()
    return nc


# --------------------------------------------------------------------------
# top-level kernel
# --------------------------------------------------------------------------

def _fold_w1(W1, a_src, a_dst, hh, cc):
    W1r = W1.reshape(W1.shape[0], hh, cc)
    ws = np.einsum("khc,hc->kh", W1r, a_src)
    wd = np.einsum("khc,hc->kh", W1r, a_dst)
    return np.concatenate([W1, ws, wd], axis=1)


RUNLOG = []  # populated when BASS_TRACE is set (test harness only)


def _run(cfg, layer, D_tbl, tile_off, CB, blobs, feat_new, Wp, bias_vec):
    """feat_new: [NV, F] float16 (rows in new-id order). Returns per-core outs."""
    from concourse.bass_utils import run_bass_kernel_spmd
    nc = build_launch(cfg, layer, D_tbl, tile_off, CB)
    fT = np.ascontiguousarray(feat_new.T)
    in_maps = []
    for r in range(cfg["NCORES"]):
        in_maps.append({
            "fT": fT, "Wp": Wp.astype(np.float16),
            "bias": np.ascontiguousarray(bias_vec, dtype=np.float32),
            "idxb": blobs[r],
        })
    res = run_bass_kernel_spmd(nc, in_maps, list(range(cfg["NCORES"])))
    if res.exec_time_ns is not None:
        tr = res.instructions_and_trace
        RUNLOG.append({"layer": layer, "exec_time_ns": res.exec_time_ns,
                       "trace": tr[1] if tr else None,
                       "profile_json": res.profile_json})
    return [res.results[r]["out"] for r in range(cfg["NCORES"])]


def kernel(x, edge_index, W1, a_src1, a_dst1, b1, W2, a_src2, a_dst2, b2):
    x = np.asarray(x)
    edge_index = np.asarray(edge_index)
    cfg = make_cfg()
    G, GSZ, SH = cfg["G"], cfg["GSZ"], cfg["SH"]
    N = cfg["N_RAW"]

    src, dst = edge_index[0], edge_index[1]
    D_tbl, blobs, orows, tile_off, CB = prep_graph(cfg, src, dst)

    orig = np.arange(N, dtype=np.int64)
    new_id = (orig % G) * GSZ + orig // G

    # launch 1
    W1p = _fold_w1(np.asarray(W1), np.asarray(a_src1), np.asarray(a_dst1),
                   cfg["HH"], cfg["CC"])
    feat1 = np.zeros((cfg["NV"], cfg["F_IN"]), dtype=np.float16)
    feat1[new_id] = x.astype(np.float16)
    outs1 = _run(cfg, 1, D_tbl, tile_off, CB, blobs, feat1, W1p,
                 np.asarray(b1))

    h2 = np.empty((N, cfg["F_HID"]), dtype=np.float32)
    for r in range(cfg["NCORES"]):
        h2[orows[r]] = outs1[r][:SH]

    # launch 2
    W2_ = np.asarray(W2)
    W2p = np.concatenate([W2_,
                          (W2_ @ np.asarray(a_src2)[0])[:, None],
                          (W2_ @ np.asarray(a_dst2)[0])[:, None]], axis=1)
    feat2 = np.zeros((cfg["NV"], cfg["F_HID"]), dtype=np.float16)
    feat2[new_id] = h2.astype(np.float16)
    outs2 = _run(cfg, 2, D_tbl, tile_off, CB, blobs, feat2, W2p,
                 np.asarray(b2))

    out = np.empty((N, cfg["OUT_W"]), dtype=np.float32)
    for r in range(cfg["NCORES"]):
        out[orows[r]] = outs2[r][:SH]
    return out



# revision 13
# speedup vs baseline: 1.8759x; 1.8759x over previous
"""Two-layer GAT on 8 Trainium2 NeuronCores — single launch, Bass/Tile.

v2 design
---------
* Table row order = dst-core-major: node n -> row = core(n)*12544 + pos(n).
  Each core's transform shard IS its own dst block, so per-dst "own row"
  data (h, a_src·h, a_dst·h) stays in SBUF — no reserved gather slots.
* Gather groups are row PHASES (row % 4) via elem_step=512 strided
  dma_gather (int16 idx = row//4 < 25088). Each node's phase is chosen by
  a greedy balancer so every dst segment has near-equal per-phase counts,
  cutting slot padding from ~2.1x to ~1.25x.
* Self-loop edges are folded in analytically from the SBUF-resident own
  rows (never gathered).
* x@W runs sharded (1/8 nodes per core); AllGather broadcasts the table;
  pad rows (positions 12500..12543 of each block) are poisoned with
  a_src = -60000 so their exp() contribution is exactly 0.
* Both layers run in ONE device launch; layer-2 table = elu(out1)@W2p is
  transposed+transformed on-device, AllGathered, and the SAME index blob
  drives both edge phases (identical graph layout).
* Gathers: 1024-idx calls, single_packet=False, 4 SWDGE queues (measured
  ~81 GB/s/core vs 42 GB/s for the default config).
"""
import sys
sys.path.insert(0, "/opt/trn_rl_repo")

import numpy as np

P = 128
NCORE = 8
N = 100000
SH = 12500            # real dst nodes per core
NT = 98
PC = NT * P           # 12544 padded positions per core
NV = PC * NCORE       # 100352 table rows
G = 4                 # phases
PH = NV // G          # 25088 rows per phase (int16-addressable)
F_IN = 512
HH1, CC1 = 8, 8
CT1 = HH1 * CC1       # 64
RW1 = CT1 + 2 * HH1   # 80: [h(64) | as(8) | ad(8)]
OUT2 = 40
RW2 = OUT2 + 2        # 42: [g(40) | as2 | ad2]
NEG = -60000.0
DCH = 8               # slot-columns per gather call (1024 idxs)
PAD_IDX = 3125        # row 12500+g = 4*3125+g  (core-0 pad rows, any phase)

RUNLOG = []           # filled when BASS_TRACE is set (test harness only)


# ==========================================================================
# host-side graph prep
# ==========================================================================

def assign_phases(es, ed):
    """Greedy phase assignment balancing per-dst-segment phase counts.

    Returns phase[N] int8 with exactly SH//G nodes per (core, phase).
    """
    out_deg = np.bincount(es, minlength=N)
    order = np.argsort(es, kind="stable")
    ed_s = ed[order]
    starts = np.zeros(N + 1, np.int64)
    np.cumsum(out_deg, out=starts[1:])

    cnt = np.zeros((N, G), np.int32)        # per dst, per phase in-counts
    cap = np.full((NCORE, G), SH // G, np.int32)
    phase = np.zeros(N, np.int8)
    BIG = np.int32(1 << 30)

    proc = np.argsort(-out_deg, kind="stable")
    for s in proc:
        dsts = ed_s[starts[s]:starts[s + 1]]
        r = s // SH
        sc = cnt[dsts].sum(axis=0, dtype=np.int64)
        sc = np.where(cap[r] > 0, sc, BIG)
        g = int(np.argmin(sc))
        phase[s] = g
        cap[r, g] -= 1
        cnt[dsts, g] += 1

    # one refinement sweep
    for s in proc:
        dsts = ed_s[starts[s]:starts[s + 1]]
        if len(dsts) == 0:
            continue
        r, g0 = s // SH, phase[s]
        sc = cnt[dsts].sum(axis=0, dtype=np.int64)
        sc[g0] -= len(dsts)
        scm = np.where((cap[r] > 0) | (np.arange(G) == g0), sc, BIG)
        g = int(np.argmin(scm))
        if g != g0:
            phase[s] = g
            cap[r, g] -= 1
            cap[r, g0] += 1
            cnt[dsts, g0] -= 1
            cnt[dsts, g] += 1
    return phase, cnt


def prep_graph(src, dst):
    """Build phases, positions, per-tile slot tables and idx blobs."""
    src = np.asarray(src, np.int64)
    dst = np.asarray(dst, np.int64)
    m = src != dst
    es, ed = src[m], dst[m]

    phase, cnt = assign_phases(es, ed)

    # positions: per core, per phase, in-degree-descending fill
    indeg = np.bincount(ed, minlength=N)
    pos = np.empty(N, np.int64)
    orows = np.full((NCORE, PC), -1, np.int64)
    for r in range(NCORE):
        nodes = np.arange(r * SH, (r + 1) * SH)
        nodes = nodes[np.argsort(-indeg[nodes], kind="stable")]
        for q in range(G):
            nq = nodes[phase[nodes] == q]
            pq = 4 * np.arange(len(nq)) + q
            pos[nq] = pq
            orows[r, pq] = nq
    row_of = (np.arange(N) // SH) * PC + pos          # table row of node

    # per-core tile max counts -> uniform D table
    D_core = np.zeros((NCORE, NT, G), np.int64)
    for r in range(NCORE):
        carr = np.zeros((PC, G), np.int32)
        nodes = np.arange(r * SH, (r + 1) * SH)
        carr[pos[nodes]] = cnt[nodes]
        D_core[r] = carr.reshape(NT, P, G).max(axis=1)
    D_tbl = D_core.max(axis=0)                        # [NT, G]

    tile_off = []
    off = 0
    for t in range(NT):
        tile_off.append(off)
        off += 8 * int(D_tbl[t].sum())
    CB = off

    # per-core idx blobs
    idxval = (row_of[es] // G).astype(np.int16)
    gval = phase[es].astype(np.int64)
    blobs = []
    dmax = int(D_tbl.max())
    for r in range(NCORE):
        mm = (ed >= r * SH) & (ed < (r + 1) * SH)
        e_pos = pos[ed[mm]]
        e_g = gval[mm]
        e_idx = idxval[mm]
        key = e_pos * G + e_g
        o = np.argsort(key, kind="stable")
        ks, vs = key[o], e_idx[o]
        cnt_pg = np.bincount(ks, minlength=PC * G)
        st = np.zeros(PC * G, np.int64)
        np.cumsum(cnt_pg[:-1], out=st[1:])
        col = np.arange(len(ks)) - st[ks]
        big = np.full((PC, G, dmax), PAD_IDX, np.int16)
        big[ks // G, ks % G, col] = vs

        blob = np.empty((P, CB), np.int16)
        for t in range(NT):
            c = tile_off[t]
            for g in range(G):
                D = int(D_tbl[t, g])
                mat = big[t * P:(t + 1) * P, g, :D]       # [128, D]
                for ck in range(0, D, DCH):
                    d = min(DCH, D - ck)
                    L = mat[:, ck:ck + d].T.ravel()       # i = col*128+part
                    W16 = L.reshape(-1, 16).T             # [16, 8d]
                    blob[:, c:c + 8 * d] = np.tile(W16, (8, 1))
                    c += 8 * d
        blobs.append(blob)

    return pos, row_of, orows, D_tbl, tile_off, CB, blobs


# ==========================================================================
# device program (single launch, both layers)
# ==========================================================================

def build_launch(D_tbl, tile_off, CB):
    from concourse import mybir
    import concourse.bass as bass
    import concourse.bacc as bacc
    import concourse.tile as tile
    import contextlib

    f16, f32, i16 = mybir.dt.float16, mybir.dt.float32, mybir.dt.int16
    AS1, AD1 = CT1, CT1 + HH1            # 64, 72
    AS2, AD2 = OUT2, OUT2 + 1            # 40, 41

    nc = bacc.Bacc("TRN2", target_bir_lowering=False, debug=False,
                   num_swdge_queues=4, num_devices=8)
    fT = nc.dram_tensor("fT", [F_IN, PC], f16, kind="ExternalInput")
    lnm = nc.dram_tensor("lnm", [P, NT], f32, kind="ExternalInput")
    Wp = nc.dram_tensor("Wp", [F_IN, RW1], f16, kind="ExternalInput")
    W2p = nc.dram_tensor("W2p", [CT1, RW2], f16, kind="ExternalInput")
    b1t = nc.dram_tensor("b1t", [CT1], f32, kind="ExternalInput")
    b2t = nc.dram_tensor("b2t", [OUT2], f32, kind="ExternalInput")
    ident = nc.dram_tensor("ident", [P, P], f16, kind="ExternalInput")
    idxb = nc.dram_tensor("idxb", [P, CB], i16, kind="ExternalInput")
    outo = nc.dram_tensor("out", [PC, OUT2], f32, kind="ExternalOutput")

    cc1 = nc.dram_tensor("cc1", [PC, 128], f16)
    cc2 = nc.dram_tensor("cc2", [PC, 128], f16)
    tb1 = nc.dram_tensor("tb1", [NV, 128], f16, addr_space="Shared")
    tb2 = nc.dram_tensor("tb2", [NV, 128], f16, addr_space="Shared")

    qn = [0]

    def nextq():
        qn[0] = (qn[0] + 1) % 4
        return qn[0]

    def bcast_load(dram_ap, w, pool, dt, tag):
        t = pool.tile([P, w], dt, tag=tag)
        ap = bass.AP(tensor=dram_ap.tensor, offset=dram_ap.offset,
                     ap=[[0, P]] + list(dram_ap.ap))
        nc.sync.dma_start(out=t[:], in_=ap)
        return t

    with tile.TileContext(nc) as tc:
        with contextlib.ExitStack() as ctx:
            singles = ctx.enter_context(tc.tile_pool(name="singles", bufs=1))
            xtp = ctx.enter_context(tc.tile_pool(name="xt", bufs=3))
            psp = ctx.enter_context(tc.tile_pool(name="ps", bufs=4, space="PSUM"))
            ps2 = ctx.enter_context(tc.tile_pool(name="ps2", bufs=2, space="PSUM"))
            gp = ctx.enter_context(tc.tile_pool(name="gp", bufs=2))
            ip = ctx.enter_context(tc.tile_pool(name="ip", bufs=2))
            ep = ctx.enter_context(tc.tile_pool(name="ep", bufs=2))
            sp = ctx.enter_context(tc.tile_pool(name="sp", bufs=3))
            mp = ctx.enter_context(tc.tile_pool(name="mp", bufs=2))
            tp2 = ctx.enter_context(tc.tile_pool(name="tp2", bufs=3))

            # ---------------- singles ----------------
            wts = []
            for kc in range(4):
                wt = singles.tile([P, RW1], f16, tag=f"w{kc}")
                nc.sync.dma_start(out=wt[:], in_=Wp[kc * P:(kc + 1) * P, :])
                wts.append(wt)
            w2sb = singles.tile([CT1, RW2], f16)
            nc.sync.dma_start(out=w2sb[:], in_=W2p[:])
            idt = singles.tile([P, P], f16)
            nc.sync.dma_start(out=idt[:], in_=ident[:])
            bt1 = bcast_load(b1t[:], CT1, singles, f32, "bt1")
            bt2 = bcast_load(b2t[:], OUT2, singles, f32, "bt2")
            # poison rows: zeros, NEG at 40:42 and 64:80
            pois = singles.tile([PC - SH, 128], f16, tag="pois")
            nc.vector.memset(pois[:], 0.0)
            nc.vector.memset(pois[:, AS2:AD2 + 1], NEG)
            nc.vector.memset(pois[:, AS1:AD1 + HH1], NEG)
            # own-row tables (SBUF-resident)
            hsb = singles.tile([P, NT, RW1], f16, tag="hsb")
            h2sb = singles.tile([P, NT, CT1], f16, tag="h2sb")
            gsb = singles.tile([P, NT, RW2], f16, tag="gsb")
            lnmt = singles.tile([P, NT], f32, tag="lnm")
            nc.sync.dma_start(out=lnmt[:], in_=lnm[:])

            # ---------------- transform 1: x@W1p for own block ----------
            chunks = [(c, min(512, PC - c)) for c in range(0, PC, 512)]
            for c0, cw in chunks:
                xts = []
                for kc in range(4):
                    xt = xtp.tile([P, 512], f16, tag=f"x{kc}")
                    nc.sync.dma_start(out=xt[:, :cw],
                                      in_=fT[kc * P:(kc + 1) * P, c0:c0 + cw])
                    xts.append(xt)
                for sub in range(cw // P):
                    ti = (c0 + sub * P) // P
                    pt = psp.tile([P, RW1], f32)
                    for kc in range(4):
                        nc.tensor.matmul(out=pt[:],
                                         lhsT=xts[kc][:, sub * P:(sub + 1) * P],
                                         rhs=wts[kc][:],
                                         start=(kc == 0), stop=(kc == 3))
                    nc.vector.tensor_copy(out=hsb[:, ti, :], in_=pt[:])
                    nc.sync.dma_start(out=cc1[ti * P:(ti + 1) * P, 0:RW1],
                                      in_=hsb[:, ti, :])

            nc.gpsimd.collective_compute(
                "AllGather", mybir.AluOpType.bypass,
                replica_groups=[list(range(NCORE))],
                ins=[cc1[:]], outs=[tb1[:]])
            tc.strict_bb_all_engine_barrier()
            for r in range(NCORE):
                nc.sync.dma_start(out=tb1[r * PC + SH:r * PC + SH + 44, :],
                                  in_=pois[:])
            tc.strict_bb_all_engine_barrier()

            # ---------------- shared edge-phase body --------------------
            def edge_tile(t, tbl, HH, CC, CT, RW, AS, AD, own, bt, is_l1):
                mybir_ = mybir
                Ds = [int(D_tbl[t, g]) for g in range(G)]
                SD = sum(Ds)
                SD1 = SD + 1                      # +1 self slot (computed)
                tcols = 8 * SD
                it = ip.tile([P, tcols], i16, tag="idx")
                nc.sync.dma_start(
                    out=it[:], in_=idxb[:, tile_off[t]:tile_off[t] + tcols])
                Gt = gp.tile([P, SD1, 128], f16, tag="G")
                c0 = 0
                ic = 0
                for g in range(G):
                    D = Ds[g]
                    nck = (D + DCH - 1) // DCH
                    base, rem = (D // nck, D % nck) if nck else (0, 0)
                    for j in range(nck):
                        d = base + (1 if j < rem else 0)
                        ap = bass.AP(tensor=tbl[:].tensor, offset=g * 128,
                                     ap=[[512, PH], [1, 128]])
                        nc.gpsimd.dma_gather(
                            out_ap=Gt[:, c0:c0 + d, :],
                            in_ap=ap,
                            idxs_ap=it[:, ic:ic + 8 * d],
                            num_idxs=P * d, num_idxs_reg=P * d,
                            elem_size=128, elem_step=512,
                            single_packet=False,
                            queue_num=nextq())
                        c0 += d
                        ic += 8 * d
                # self slot: own row (h | as | ad)
                nc.vector.tensor_copy(out=Gt[:, SD, 0:RW], in_=own[:])

                # e = leaky(as_src + ad_own)   [P, SD1, HH] f16
                et = ep.tile([P, SD1, HH], f16, tag="e")
                nc.vector.tensor_tensor(
                    out=et[:],
                    in0=Gt[:, :, AS:AS + HH],
                    in1=own[:, AD:AD + HH].unsqueeze(1)
                        .broadcast_to([P, SD1, HH]),
                    op=mybir_.AluOpType.add)
                ef = et[:].rearrange("p d h -> p (d h)")
                nc.scalar.activation(out=ef, in_=ef,
                                     func=mybir_.ActivationFunctionType.Prelu,
                                     alpha=0.2)
                # self-loop multiplicity: e_self += ln(mult)
                nc.vector.tensor_scalar_add(out=et[:, SD, :],
                                            in0=et[:, SD, :],
                                            scalar1=lnmt[:, t:t + 1])
                # -max, subtract, exp
                nmt = sp.tile([P, HH], f32, tag="nm")
                nc.vector.reduce_max(out=nmt[:],
                                     in_=et[:].rearrange("p d h -> p h d"),
                                     axis=mybir_.AxisListType.X, negate=True)
                nc.vector.tensor_tensor(
                    out=et[:], in0=et[:],
                    in1=nmt[:].unsqueeze(1).broadcast_to([P, SD1, HH]),
                    op=mybir_.AluOpType.add)
                ext = ep.tile([P, SD1, HH], f16, tag="ex")
                nc.scalar.activation(out=ext[:].rearrange("p d h -> p (d h)"),
                                     in_=ef,
                                     func=mybir_.ActivationFunctionType.Exp)

                # denom + reciprocal
                dent = sp.tile([P, HH], f32, tag="den")
                nc.vector.reduce_sum(out=dent[:],
                                     in_=ext[:].rearrange("p d h -> p h d"),
                                     axis=mybir_.AxisListType.X)
                rdt = sp.tile([P, HH], f32, tag="rd")
                nc.vector.reciprocal(out=rdt[:], in_=dent[:])

                # weighted message sum (f16 products, f32 accumulate)
                mg = mp.tile([P, SD1, CT], f16, tag="msg")
                nc.vector.tensor_tensor(
                    out=mg[:].rearrange("p d (h c) -> p d h c", h=HH),
                    in0=Gt[:, :, 0:CT].rearrange("p d (h c) -> p d h c", h=HH),
                    in1=ext[:].unsqueeze(3).broadcast_to([P, SD1, HH, CC]),
                    op=mybir_.AluOpType.mult)
                ort = sp.tile([P, CT], f32, tag="or")
                nc.vector.reduce_sum(out=ort[:],
                                     in_=mg[:].rearrange("p d f -> p f d"),
                                     axis=mybir_.AxisListType.X)

                # normalize + bias
                o1 = sp.tile([P, CT], f32, tag="o1")
                nc.vector.tensor_tensor(
                    out=o1[:].rearrange("p (h c) -> p h c", h=HH),
                    in0=ort[:].rearrange("p (h c) -> p h c", h=HH),
                    in1=rdt[:].unsqueeze(2).broadcast_to([P, HH, CC]),
                    op=mybir_.AluOpType.mult)
                nc.vector.tensor_add(out=o1[:], in0=o1[:], in1=bt[:])

                if is_l1:
                    # elu(x) = relu(x) + exp(min(x,0)) - 1
                    t1 = sp.tile([P, CT], f32, tag="t1")
                    nc.vector.tensor_scalar_min(out=t1[:], in0=o1[:], scalar1=0.0)
                    nc.scalar.activation(out=t1[:], in_=t1[:],
                                         func=mybir_.ActivationFunctionType.Exp)
                    t2 = sp.tile([P, CT], f32, tag="t2")
                    nc.scalar.activation(out=t2[:], in_=o1[:],
                                         func=mybir_.ActivationFunctionType.Relu)
                    nc.vector.tensor_add(out=t1[:], in0=t1[:], in1=t2[:])
                    nc.vector.tensor_scalar_add(out=h2sb[:, t, :], in0=t1[:],
                                                scalar1=-1.0)
                else:
                    nc.sync.dma_start(out=outo[t * P:(t + 1) * P, :], in_=o1[:])

            # ---------------- layer-1 edge phase + transform 2 ----------
            for t in range(NT):
                edge_tile(t, tb1, HH1, CC1, CT1, RW1, AS1, AD1,
                          hsb[:, t, :], bt1, True)
                # transform2 for this tile: g = elu_h2 @ W2p
                pT = ps2.tile([CT1, P], f32, tag="pT")
                nc.tensor.matmul(out=pT[:], lhsT=h2sb[:, t, :], rhs=idt[:],
                                 start=True, stop=True)
                tsb = tp2.tile([CT1, P], f16, tag="tsb")
                nc.vector.tensor_copy(out=tsb[:], in_=pT[:])
                p2 = ps2.tile([P, RW2], f32, tag="p2")
                nc.tensor.matmul(out=p2[:], lhsT=tsb[:], rhs=w2sb[:],
                                 start=True, stop=True)
                nc.vector.tensor_copy(out=gsb[:, t, :], in_=p2[:])
                nc.sync.dma_start(out=cc2[t * P:(t + 1) * P, 0:RW2],
                                  in_=gsb[:, t, :])

            tc.strict_bb_all_engine_barrier()
            nc.gpsimd.collective_compute(
                "AllGather", mybir.AluOpType.bypass,
                replica_groups=[list(range(NCORE))],
                ins=[cc2[:]], outs=[tb2[:]])
            tc.strict_bb_all_engine_barrier()
            for r in range(NCORE):
                nc.sync.dma_start(out=tb2[r * PC + SH:r * PC + SH + 44, :],
                                  in_=pois[:])
            tc.strict_bb_all_engine_barrier()

            # ---------------- layer-2 edge phase ------------------------
            for t in range(NT):
                edge_tile(t, tb2, 1, OUT2, OUT2, RW2, AS2, AD2,
                          gsb[:, t, :], bt2, False)

    nc.compile()
    return nc


# ==========================================================================
# top-level kernel
# ==========================================================================

def _fold_w1(W1, a_src, a_dst):
    W1r = W1.reshape(F_IN, HH1, CC1)
    ws = np.einsum("khc,hc->kh", W1r, a_src)
    wd = np.einsum("khc,hc->kh", W1r, a_dst)
    return np.concatenate([W1, ws, wd], axis=1)


def kernel(x, edge_index, W1, a_src1, a_dst1, b1, W2, a_src2, a_dst2, b2):
    from concourse.bass_utils import run_bass_kernel_spmd

    x = np.asarray(x)
    src, dst = np.asarray(edge_index[0]), np.asarray(edge_index[1])
    pos, row_of, orows, D_tbl, tile_off, CB, blobs = prep_graph(src, dst)

    nc = build_launch(D_tbl, tile_off, CB)

    # self-loop multiplicity: 1 (added loop) + natural src==dst edges
    selfc = np.bincount(dst[src == dst], minlength=N)
    lnmult = np.log1p(selfc.astype(np.float64)).astype(np.float32)

    W1p = _fold_w1(np.asarray(W1), np.asarray(a_src1),
                   np.asarray(a_dst1)).astype(np.float16)
    W2_ = np.asarray(W2)
    W2p = np.concatenate([W2_,
                          (W2_ @ np.asarray(a_src2)[0])[:, None],
                          (W2_ @ np.asarray(a_dst2)[0])[:, None]],
                         axis=1).astype(np.float16)
    ident = np.eye(P, dtype=np.float16)
    b1v = np.ascontiguousarray(b1, np.float32)
    b2v = np.ascontiguousarray(b2, np.float32)

    in_maps = []
    for r in range(NCORE):
        nodes = np.arange(r * SH, (r + 1) * SH)
        xs = np.zeros((PC, F_IN), np.float16)
        xs[pos[nodes]] = x[nodes].astype(np.float16)
        lv = np.zeros(PC, np.float32)
        lv[pos[nodes]] = lnmult[nodes]
        in_maps.append({
            "fT": np.ascontiguousarray(xs.T),
            "Wp": W1p, "W2p": W2p, "b1t": b1v, "b2t": b2v,
            "ident": ident, "idxb": blobs[r],
            "lnm": np.ascontiguousarray(lv.reshape(NT, P).T),
        })

    res = run_bass_kernel_spmd(nc, in_maps, list(range(NCORE)))
    if res.exec_time_ns is not None:
        tr = res.instructions_and_trace
        RUNLOG.append({"layer": "fused", "exec_time_ns": res.exec_time_ns,
                       "trace": tr[1] if tr else None,
                       "profile_json": res.profile_json})

    out = np.empty((N, OUT2), np.float32)
    for r in range(NCORE):
        valid = orows[r] >= 0
        out[orows[r][valid]] = res.results[r]["out"][valid]
    return out


# revision 21
# speedup vs baseline: 2.1839x; 1.1642x over previous
"""Two-layer GAT on 8 Trainium2 NeuronCores — single launch, Bass/Tile.

v2 design
---------
* Table row order = dst-core-major: node n -> row = core(n)*12544 + pos(n).
  Each core's transform shard IS its own dst block, so per-dst "own row"
  data (h, a_src·h, a_dst·h) stays in SBUF — no reserved gather slots.
* Gather groups are row PHASES (row % 4) via elem_step=512 strided
  dma_gather (int16 idx = row//4 < 25088). Each node's phase is chosen by
  a greedy balancer so every dst segment has near-equal per-phase counts,
  cutting slot padding from ~2.1x to ~1.25x.
* Self-loop edges are folded in analytically from the SBUF-resident own
  rows (never gathered).
* x@W runs sharded (1/8 nodes per core); AllGather broadcasts the table;
  pad rows (positions 12500..12543 of each block) are poisoned with
  a_src = -60000 so their exp() contribution is exactly 0.
* Both layers run in ONE device launch; layer-2 table = elu(out1)@W2p is
  transposed+transformed on-device, AllGathered, and the SAME index blob
  drives both edge phases (identical graph layout).
* Gathers: 1024-idx calls, single_packet=False, 4 SWDGE queues (measured
  ~81 GB/s/core vs 42 GB/s for the default config).
"""
import sys
sys.path.insert(0, "/opt/trn_rl_repo")

import numpy as np

P = 128
NCORE = 8
N = 100000
SH = 12500            # real dst nodes per core
NT = 98
PC = NT * P           # 12544 padded positions per core
NV = PC * NCORE       # 100352 table rows
G = 4                 # phases
PH = NV // G          # 25088 rows per phase (int16-addressable)
F_IN = 512
HH1, CC1 = 8, 8
CT1 = HH1 * CC1       # 64
RW1 = CT1 + 2 * HH1   # 80: [h(64) | as(8) | ad(8)]
OUT2 = 40
RW2 = OUT2 + 2        # 42: [g(40) | as2 | ad2]
NEG = -60000.0
DCH = 8               # slot-columns per gather call (1024 idxs)
PAD_IDX = 3125        # row 12500+g = 4*3125+g  (core-0 pad rows, any phase)

RUNLOG = []           # filled when BASS_TRACE is set (test harness only)


# ==========================================================================
# host-side graph prep
# ==========================================================================

def assign_phases(es, ed, core_id):
    """Greedy phase assignment balancing per-dst-segment phase counts.

    Returns phase[N] int8 with exactly SH//G nodes per (core, phase).
    """
    out_deg = np.bincount(es, minlength=N)
    order = np.argsort(es, kind="stable")
    ed_s = ed[order]
    starts = np.zeros(N + 1, np.int64)
    np.cumsum(out_deg, out=starts[1:])

    cnt = np.zeros((N, G), np.int32)        # per dst, per phase in-counts
    cap = np.full((NCORE, G), SH // G, np.int32)
    phase = np.zeros(N, np.int8)
    BIG = np.int32(1 << 30)
    gidx = np.arange(G)

    proc = np.argsort(-out_deg, kind="stable")
    for s in proc:
        dsts = ed_s[starts[s]:starts[s + 1]]
        r = core_id[s]
        sc = cnt[dsts].sum(axis=0, dtype=np.int64)
        sc = np.where(cap[r] > 0, sc, BIG)
        g = int(np.argmin(sc))
        phase[s] = g
        cap[r, g] -= 1
        cnt[dsts, g] += 1

    for _ in range(3):                       # refinement sweeps
        moved = 0
        for s in proc:
            dsts = ed_s[starts[s]:starts[s + 1]]
            if len(dsts) == 0:
                continue
            r, g0 = core_id[s], phase[s]
            sc = cnt[dsts].sum(axis=0, dtype=np.int64)
            sc[g0] -= len(dsts)
            scm = np.where((cap[r] > 0) | (gidx == g0), sc, BIG)
            g = int(np.argmin(scm))
            if g != g0:
                phase[s] = g
                cap[r, g] -= 1
                cap[r, g0] += 1
                cnt[dsts, g0] -= 1
                cnt[dsts, g] += 1
                moved += 1
        if moved == 0:
            break
    return phase, cnt


def prep_graph(src, dst):
    """Build phases, positions, per-tile slot tables and idx blobs."""
    src = np.asarray(src, np.int64)
    dst = np.asarray(dst, np.int64)
    m = src != dst
    es, ed = src[m], dst[m]

    # dst -> core: deal by in-degree so every core sees the same profile
    indeg = np.bincount(ed, minlength=N)
    by_deg = np.argsort(-indeg, kind="stable")
    core_id = np.empty(N, np.int32)
    core_id[by_deg] = np.arange(N) % NCORE

    phase, cnt = assign_phases(es, ed, core_id)

    # positions: per core, per phase, in-degree-descending fill
    pos = np.empty(N, np.int64)
    orows = np.full((NCORE, PC), -1, np.int64)
    for r in range(NCORE):
        nodes = np.where(core_id == r)[0]
        nodes = nodes[np.argsort(-indeg[nodes], kind="stable")]
        for q in range(G):
            nq = nodes[phase[nodes] == q]
            pq = 4 * np.arange(len(nq)) + q
            pos[nq] = pq
            orows[r, pq] = nq
    row_of = core_id.astype(np.int64) * PC + pos      # table row of node

    # per-core tile max counts -> uniform D table
    D_core = np.zeros((NCORE, NT, G), np.int64)
    for r in range(NCORE):
        carr = np.zeros((PC, G), np.int32)
        nodes = np.where(core_id == r)[0]
        carr[pos[nodes]] = cnt[nodes]
        D_core[r] = carr.reshape(NT, P, G).max(axis=1)
    D_tbl = D_core.max(axis=0)                        # [NT, G]

    tile_off = []
    off = 0
    for t in range(NT):
        tile_off.append(off)
        off += 8 * int(D_tbl[t].sum())
    CB = off

    # per-core idx blobs
    idxval = (row_of[es] // G).astype(np.int16)
    gval = phase[es].astype(np.int64)
    blobs = []
    dmax = int(D_tbl.max())
    for r in range(NCORE):
        mm = core_id[ed] == r
        e_pos = pos[ed[mm]]
        e_g = gval[mm]
        e_idx = idxval[mm]
        key = e_pos * G + e_g
        o = np.argsort(key, kind="stable")
        ks, vs = key[o], e_idx[o]
        cnt_pg = np.bincount(ks, minlength=PC * G)
        st = np.zeros(PC * G, np.int64)
        np.cumsum(cnt_pg[:-1], out=st[1:])
        col = np.arange(len(ks)) - st[ks]
        big = np.full((PC, G, dmax), PAD_IDX, np.int16)
        big[ks // G, ks % G, col] = vs

        blob = np.empty((P, CB), np.int16)
        for t in range(NT):
            c = tile_off[t]
            for g in range(G):
                D = int(D_tbl[t, g])
                mat = big[t * P:(t + 1) * P, g, :D]       # [128, D]
                for ck in range(0, D, DCH):
                    d = min(DCH, D - ck)
                    L = mat[:, ck:ck + d].T.ravel()       # i = col*128+part
                    W16 = L.reshape(-1, 16).T             # [16, 8d]
                    blob[:, c:c + 8 * d] = np.tile(W16, (8, 1))
                    c += 8 * d
        blobs.append(blob)

    return pos, core_id, orows, D_tbl, tile_off, CB, blobs


# ==========================================================================
# device program (single launch, both layers)
# ==========================================================================

def build_launch(D_tbl, tile_off, CB):
    from concourse import mybir
    import concourse.bass as bass
    import concourse.bacc as bacc
    import concourse.tile as tile
    import contextlib

    f16, f32, i16 = mybir.dt.float16, mybir.dt.float32, mybir.dt.int16
    AS1, AD1 = CT1, CT1 + HH1            # 64, 72
    AS2, AD2 = OUT2, OUT2 + 1            # 40, 41

    nc = bacc.Bacc("TRN2", target_bir_lowering=False, debug=False,
                   num_swdge_queues=4, num_devices=8)
    fT = nc.dram_tensor("fT", [F_IN, PC], f16, kind="ExternalInput")
    lnm = nc.dram_tensor("lnm", [P, NT], f32, kind="ExternalInput")
    Wp = nc.dram_tensor("Wp", [F_IN, RW1], f16, kind="ExternalInput")
    W2p = nc.dram_tensor("W2p", [CT1, RW2], f16, kind="ExternalInput")
    b1t = nc.dram_tensor("b1t", [CT1], f32, kind="ExternalInput")
    b2t = nc.dram_tensor("b2t", [OUT2], f32, kind="ExternalInput")
    ident = nc.dram_tensor("ident", [P, P], f16, kind="ExternalInput")
    idxb = nc.dram_tensor("idxb", [P, CB], i16, kind="ExternalInput")
    outo = nc.dram_tensor("out", [PC, OUT2], f32, kind="ExternalOutput")

    cc1 = nc.dram_tensor("cc1", [PC, 128], f16)
    cc2 = nc.dram_tensor("cc2", [PC, 128], f16)
    tb1 = nc.dram_tensor("tb1", [NV, 128], f16, addr_space="Shared")
    tb2 = nc.dram_tensor("tb2", [NV, 128], f16, addr_space="Shared")

    qn = [0]

    def nextq():
        qn[0] = (qn[0] + 1) % 4
        return qn[0]

    def bcast_load(dram_ap, w, pool, dt, tag):
        t = pool.tile([P, w], dt, tag=tag)
        ap = bass.AP(tensor=dram_ap.tensor, offset=dram_ap.offset,
                     ap=[[0, P]] + list(dram_ap.ap))
        nc.sync.dma_start(out=t[:], in_=ap)
        return t

    with tile.TileContext(nc) as tc:
        with contextlib.ExitStack() as ctx:
            singles = ctx.enter_context(tc.tile_pool(name="singles", bufs=1))
            xtp = ctx.enter_context(tc.tile_pool(name="xt", bufs=3))
            psp = ctx.enter_context(tc.tile_pool(name="ps", bufs=4, space="PSUM"))
            ps2 = ctx.enter_context(tc.tile_pool(name="ps2", bufs=2, space="PSUM"))
            gp = ctx.enter_context(tc.tile_pool(name="gp", bufs=3))
            ip = ctx.enter_context(tc.tile_pool(name="ip", bufs=3))
            ep = ctx.enter_context(tc.tile_pool(name="ep", bufs=3))
            sp = ctx.enter_context(tc.tile_pool(name="sp", bufs=4))
            mp = ctx.enter_context(tc.tile_pool(name="mp", bufs=3))
            tp2 = ctx.enter_context(tc.tile_pool(name="tp2", bufs=3))

            # ---------------- singles ----------------
            wts = []
            for kc in range(4):
                wt = singles.tile([P, RW1], f16, tag=f"w{kc}")
                nc.sync.dma_start(out=wt[:], in_=Wp[kc * P:(kc + 1) * P, :])
                wts.append(wt)
            w2sb = singles.tile([CT1, RW2], f16)
            nc.sync.dma_start(out=w2sb[:], in_=W2p[:])
            idt = singles.tile([P, P], f16)
            nc.sync.dma_start(out=idt[:], in_=ident[:])
            bt1 = bcast_load(b1t[:], CT1, singles, f32, "bt1")
            bt2 = bcast_load(b2t[:], OUT2, singles, f32, "bt2")
            # poison rows: zeros, NEG at 40:42 and 64:80
            pois = singles.tile([PC - SH, 128], f16, tag="pois")
            nc.vector.memset(pois[:], 0.0)
            nc.vector.memset(pois[:, AS2:AD2 + 1], NEG)
            nc.vector.memset(pois[:, AS1:AD1 + HH1], NEG)
            # own-row tables (SBUF-resident)
            hsb = singles.tile([P, NT, RW1], f16, tag="hsb")
            h2sb = singles.tile([P, NT, CT1], f16, tag="h2sb")
            gsb = singles.tile([P, NT, RW2], f16, tag="gsb")
            lnmt = singles.tile([P, NT], f32, tag="lnm")
            nc.sync.dma_start(out=lnmt[:], in_=lnm[:])

            # ---------------- transform 1: x@W1p for own block ----------
            chunks = [(c, min(512, PC - c)) for c in range(0, PC, 512)]
            for c0, cw in chunks:
                xts = []
                for kc in range(4):
                    xt = xtp.tile([P, 512], f16, tag=f"x{kc}")
                    nc.sync.dma_start(out=xt[:, :cw],
                                      in_=fT[kc * P:(kc + 1) * P, c0:c0 + cw])
                    xts.append(xt)
                for sub in range(cw // P):
                    ti = (c0 + sub * P) // P
                    pt = psp.tile([P, RW1], f32)
                    for kc in range(4):
                        nc.tensor.matmul(out=pt[:],
                                         lhsT=xts[kc][:, sub * P:(sub + 1) * P],
                                         rhs=wts[kc][:],
                                         start=(kc == 0), stop=(kc == 3))
                    nc.vector.tensor_copy(out=hsb[:, ti, :], in_=pt[:])
                    nc.sync.dma_start(out=cc1[ti * P:(ti + 1) * P, 0:RW1],
                                      in_=hsb[:, ti, :])

            nc.gpsimd.collective_compute(
                "AllGather", mybir.AluOpType.bypass,
                replica_groups=[list(range(NCORE))],
                ins=[cc1[:]], outs=[tb1[:]])
            tc.strict_bb_all_engine_barrier()
            for r in range(NCORE):
                nc.sync.dma_start(out=tb1[r * PC + SH:r * PC + SH + 44, :],
                                  in_=pois[:])
            tc.strict_bb_all_engine_barrier()

            # ---------------- shared edge-phase body --------------------
            def edge_tile(t, tbl, HH, CC, CT, RW, AS, AD, own, bt, is_l1):
                mybir_ = mybir
                Ds = [int(D_tbl[t, g]) for g in range(G)]
                SD = sum(Ds)
                SD1 = SD + 1                      # +1 self slot (computed)
                tcols = 8 * SD
                it = ip.tile([P, tcols], i16, tag="idx")
                nc.sync.dma_start(
                    out=it[:], in_=idxb[:, tile_off[t]:tile_off[t] + tcols])
                Gt = gp.tile([P, SD1, 128], f16, tag="G")
                c0 = 0
                ic = 0
                for g in range(G):
                    D = Ds[g]
                    nck = (D + DCH - 1) // DCH
                    base, rem = (D // nck, D % nck) if nck else (0, 0)
                    for j in range(nck):
                        d = base + (1 if j < rem else 0)
                        ap = bass.AP(tensor=tbl[:].tensor, offset=g * 128,
                                     ap=[[512, PH], [1, 128]])
                        nc.gpsimd.dma_gather(
                            out_ap=Gt[:, c0:c0 + d, :],
                            in_ap=ap,
                            idxs_ap=it[:, ic:ic + 8 * d],
                            num_idxs=P * d, num_idxs_reg=P * d,
                            elem_size=128, elem_step=512,
                            single_packet=False,
                            queue_num=nextq())
                        c0 += d
                        ic += 8 * d
                # self slot: own row (h | as | ad)
                nc.vector.tensor_copy(out=Gt[:, SD, 0:RW], in_=own[:])

                # e = leaky(as_src + ad_own)   [P, SD1, HH] f32
                et = ep.tile([P, SD1, HH], f32, tag="e")
                nc.vector.tensor_tensor(
                    out=et[:],
                    in0=Gt[:, :, AS:AS + HH],
                    in1=own[:, AD:AD + HH].unsqueeze(1)
                        .broadcast_to([P, SD1, HH]),
                    op=mybir_.AluOpType.add)
                ef = et[:].rearrange("p d h -> p (d h)")
                nc.scalar.activation(out=ef, in_=ef,
                                     func=mybir_.ActivationFunctionType.Prelu,
                                     alpha=0.2)
                # self-loop multiplicity: e_self += ln(mult)
                nc.vector.tensor_scalar_add(out=et[:, SD, :],
                                            in0=et[:, SD, :],
                                            scalar1=lnmt[:, t:t + 1])
                # -max, subtract, exp
                nmt = sp.tile([P, HH], f32, tag="nm")
                nc.vector.reduce_max(out=nmt[:],
                                     in_=et[:].rearrange("p d h -> p h d"),
                                     axis=mybir_.AxisListType.X, negate=True)
                nc.vector.tensor_tensor(
                    out=et[:], in0=et[:],
                    in1=nmt[:].unsqueeze(1).broadcast_to([P, SD1, HH]),
                    op=mybir_.AluOpType.add)
                ext = ep.tile([P, SD1, HH], f16, tag="ex")
                nc.scalar.activation(out=ext[:].rearrange("p d h -> p (d h)"),
                                     in_=ef,
                                     func=mybir_.ActivationFunctionType.Exp)

                # denom + reciprocal
                dent = sp.tile([P, HH], f32, tag="den")
                nc.vector.reduce_sum(out=dent[:],
                                     in_=ext[:].rearrange("p d h -> p h d"),
                                     axis=mybir_.AxisListType.X)
                rdt = sp.tile([P, HH], f32, tag="rd")
                nc.vector.reciprocal(out=rdt[:], in_=dent[:])

                # weighted message sum (f16 products, f32 accumulate)
                mg = mp.tile([P, SD1, CT], f16, tag="msg")
                nc.vector.tensor_tensor(
                    out=mg[:].rearrange("p d (h c) -> p d h c", h=HH),
                    in0=Gt[:, :, 0:CT].rearrange("p d (h c) -> p d h c", h=HH),
                    in1=ext[:].unsqueeze(3).broadcast_to([P, SD1, HH, CC]),
                    op=mybir_.AluOpType.mult)
                ort = sp.tile([P, CT], f32, tag="or")
                nc.vector.reduce_sum(out=ort[:],
                                     in_=mg[:].rearrange("p d f -> p f d"),
                                     axis=mybir_.AxisListType.X)

                # normalize + bias
                o1 = sp.tile([P, CT], f32, tag="o1")
                nc.vector.tensor_tensor(
                    out=o1[:].rearrange("p (h c) -> p h c", h=HH),
                    in0=ort[:].rearrange("p (h c) -> p h c", h=HH),
                    in1=rdt[:].unsqueeze(2).broadcast_to([P, HH, CC]),
                    op=mybir_.AluOpType.mult)
                nc.vector.tensor_add(out=o1[:], in0=o1[:], in1=bt[:])

                if is_l1:
                    # elu(x) = relu(x) + exp(min(x,0)) - 1
                    t1 = sp.tile([P, CT], f32, tag="t1")
                    nc.vector.tensor_scalar_min(out=t1[:], in0=o1[:], scalar1=0.0)
                    nc.scalar.activation(out=t1[:], in_=t1[:],
                                         func=mybir_.ActivationFunctionType.Exp)
                    t2 = sp.tile([P, CT], f32, tag="t2")
                    nc.scalar.activation(out=t2[:], in_=o1[:],
                                         func=mybir_.ActivationFunctionType.Relu)
                    nc.vector.tensor_add(out=t1[:], in0=t1[:], in1=t2[:])
                    nc.vector.tensor_scalar_add(out=h2sb[:, t, :], in0=t1[:],
                                                scalar1=-1.0)
                else:
                    nc.sync.dma_start(out=outo[t * P:(t + 1) * P, :], in_=o1[:])

            # ---------------- layer-1 edge phase + transform 2 ----------
            for t in range(NT):
                edge_tile(t, tb1, HH1, CC1, CT1, RW1, AS1, AD1,
                          hsb[:, t, :], bt1, True)
                # transform2 for this tile: g = elu_h2 @ W2p
                pT = ps2.tile([CT1, P], f32, tag="pT")
                nc.tensor.matmul(out=pT[:], lhsT=h2sb[:, t, :], rhs=idt[:],
                                 start=True, stop=True)
                tsb = tp2.tile([CT1, P], f16, tag="tsb")
                nc.vector.tensor_copy(out=tsb[:], in_=pT[:])
                p2 = ps2.tile([P, RW2], f32, tag="p2")
                nc.tensor.matmul(out=p2[:], lhsT=tsb[:], rhs=w2sb[:],
                                 start=True, stop=True)
                nc.vector.tensor_copy(out=gsb[:, t, :], in_=p2[:])
                nc.sync.dma_start(out=cc2[t * P:(t + 1) * P, 0:RW2],
                                  in_=gsb[:, t, :])

            tc.strict_bb_all_engine_barrier()
            nc.gpsimd.collective_compute(
                "AllGather", mybir.AluOpType.bypass,
                replica_groups=[list(range(NCORE))],
                ins=[cc2[:]], outs=[tb2[:]])
            tc.strict_bb_all_engine_barrier()
            for r in range(NCORE):
                nc.sync.dma_start(out=tb2[r * PC + SH:r * PC + SH + 44, :],
                                  in_=pois[:])
            tc.strict_bb_all_engine_barrier()

            # ---------------- layer-2 edge phase ------------------------
            for t in range(NT):
                edge_tile(t, tb2, 1, OUT2, OUT2, RW2, AS2, AD2,
                          gsb[:, t, :], bt2, False)

    nc.compile()
    return nc


# ==========================================================================
# top-level kernel
# ==========================================================================

def _fold_w1(W1, a_src, a_dst):
    W1r = W1.reshape(F_IN, HH1, CC1)
    ws = np.einsum("khc,hc->kh", W1r, a_src)
    wd = np.einsum("khc,hc->kh", W1r, a_dst)
    return np.concatenate([W1, ws, wd], axis=1)


def kernel(x, edge_index, W1, a_src1, a_dst1, b1, W2, a_src2, a_dst2, b2):
    from concourse.bass_utils import run_bass_kernel_spmd

    x = np.asarray(x)
    src, dst = np.asarray(edge_index[0]), np.asarray(edge_index[1])
    pos, core_id, orows, D_tbl, tile_off, CB, blobs = prep_graph(src, dst)

    nc = build_launch(D_tbl, tile_off, CB)

    # self-loop multiplicity: 1 (added loop) + natural src==dst edges
    selfc = np.bincount(dst[src == dst], minlength=N)
    lnmult = np.log1p(selfc.astype(np.float64)).astype(np.float32)

    W1p = _fold_w1(np.asarray(W1), np.asarray(a_src1),
                   np.asarray(a_dst1)).astype(np.float16)
    W2_ = np.asarray(W2)
    W2p = np.concatenate([W2_,
                          (W2_ @ np.asarray(a_src2)[0])[:, None],
                          (W2_ @ np.asarray(a_dst2)[0])[:, None]],
                         axis=1).astype(np.float16)
    ident = np.eye(P, dtype=np.float16)
    b1v = np.ascontiguousarray(b1, np.float32)
    b2v = np.ascontiguousarray(b2, np.float32)

    in_maps = []
    for r in range(NCORE):
        nodes = np.where(core_id == r)[0]
        xs = np.zeros((PC, F_IN), np.float16)
        xs[pos[nodes]] = x[nodes].astype(np.float16)
        lv = np.zeros(PC, np.float32)
        lv[pos[nodes]] = lnmult[nodes]
        in_maps.append({
            "fT": np.ascontiguousarray(xs.T),
            "Wp": W1p, "W2p": W2p, "b1t": b1v, "b2t": b2v,
            "ident": ident, "idxb": blobs[r],
            "lnm": np.ascontiguousarray(lv.reshape(NT, P).T),
        })

    res = run_bass_kernel_spmd(nc, in_maps, list(range(NCORE)))
    if res.exec_time_ns is not None:
        tr = res.instructions_and_trace
        RUNLOG.append({"layer": "fused", "exec_time_ns": res.exec_time_ns,
                       "trace": tr[1] if tr else None,
                       "profile_json": res.profile_json})

    out = np.empty((N, OUT2), np.float32)
    for r in range(NCORE):
        valid = orows[r] >= 0
        out[orows[r][valid]] = res.results[r]["out"][valid]
    return out
